# revision 4
# baseline (speedup 1.0000x reference)
"""Trainium2 Bass kernel for nn_Decoder (video-caption decoder step).

Sharding: data-parallel over batch (128 rows/core) for attention+LSTM;
vocab-parallel (6283 cols/core) for the logit projection + log-softmax,
with an AllGather of h^T between the phases and AllReduce of softmax sums.
"""

import os
import sys

sys.path.insert(0, "/opt/trn_rl_repo")
sys.path.insert(0, "/opt/pypackages")

import numpy as np
import ml_dtypes

import concourse.bass as bass
import concourse.bacc as bacc
import concourse.tile as tile
import concourse.mybir as mybir
from concourse import bass_utils

BF16 = mybir.dt.bfloat16
F32 = mybir.dt.float32
F32R = mybir.dt.float32r
AF = mybir.ActivationFunctionType
OP = mybir.AluOpType

B, H, S, E, V = 1024, 512, 300, 512, 50257
NCORES = 8
BL = B // NCORES          # 128 batch rows per core
VL = 6283                 # vocab cols per core (8*6283 = 50264 >= V)
VPAD = VL * NCORES
NEG_BIG = -1e30

# attention configs: (name, N, D, ksizes, feats_slot)
ATT = [
    ("obj", 36, 512, [128, 128, 128, 128], "A"),
    ("mot", 26, 512, [128, 128, 128, 128], "B"),
    ("vid", 26, 512, [128, 128, 128, 128], "A"),
    ("oss", 20, 300, [128, 128, 44], "B"),
]

N_TILES = [(i * 512, min(512, VL - i * 512)) for i in range((VL + 511) // 512)]
OCH = [(o, min(1048, VL - o)) for o in range(0, VL, 1048)]

DEBUG_TAPS = bool(int(os.environ.get("KBD_TAPS", "0")))


def _ceil(a, b):
    return (a + b - 1) // b


def build_program():
    nc = bacc.Bacc("TRN2", target_bir_lowering=False, debug=False,
                   num_devices=NCORES)

    def din(name, shape, dt):
        return nc.dram_tensor(name, shape, dt, kind="ExternalInput")

    def dout(name, shape, dt):
        return nc.dram_tensor(name, shape, dt, kind="ExternalOutput")

    feats_d = {
        "obj": din("obj", [BL, 36, 512], F32),
        "mot": din("mot", [BL, 26, 512], F32),
        "vid": din("vid", [BL, 26, 512], F32),
        "oss": din("oss", [BL, 20, 300], F32),
    }
    asem_d = din("asem", [BL, S], F32)
    vsem_d = din("vsem", [BL, S], F32)
    embed_d = din("embed", [BL, E], F32)
    h0_d = din("h0", [BL, H], F32)
    c0_d = din("c0", [BL, H], F32)

    ident_f_d = din("ident_f", [128, 128], F32)
    ident_b_d = din("ident_b", [128, 128], BF16)
    ones_f_d = din("ones_f", [1, 128], F32R)
    ones_b_d = din("ones_b", [1, 128], BF16)

    wU_d = {
        "obj": din("wUo", [512, 512], BF16),
        "mot": din("wUm", [512, 512], BF16),
        "vid": din("wUv", [512, 512], BF16),
        "oss": din("wUos", [384, 512], BF16),   # zero-padded 300->384
    }
    wWw_d = din("wWw", [512, 512], BF16)        # W_w.T
    attb_d = din("attb", [4, 512], BF16)        # bo,bm,bv,bos rows
    attw_d = din("attw", [4, 512], BF16)        # wo,wm,wv,wos rows
    wVis_d = din("wVis", [1536, 512], F32R)      # vis_w.T
    visb_d = din("visb", [1, 512], F32R)
    wSem_d = din("wSem", [1152, 512], F32R)      # sem_w.T seg-padded (3x384)
    semb_d = din("semb", [1, 512], F32R)
    wIh_d = din("wIh", [1536, 2048], F32R)       # W_ih.T
    wHh_d = din("wHh", [512, 2048], F32R)        # W_hh.T
    gbias_d = din("gbias", [4, 512], F32R)       # (b_ih+b_hh) as 4 rows
    wTw_d = din("wTw", [512, 512], BF16)        # to_word_w.T [H,E]
    twb_d = din("twb", [128, 4], F32)           # to_word_b col-major tiles
    wLg_d = din("wLg", [512, VL], BF16)         # logit_w shard transposed
    lb_d = din("lb", [1, VL], BF16)             # logit_b shard (pad=-1e30)

    out_logp = dout("out_logp", [B, VL], F32)
    out_h = dout("out_h", [BL, H], F32)
    out_c = dout("out_c", [BL, H], F32)
    taps = {}
    if DEBUG_TAPS:
        taps["t_attn"] = dout("t_attn", [BL, 4, 512], F32)
        taps["t_scores"] = dout("t_scores", [BL, 4, 36], F32)
        taps["t_x"] = dout("t_x", [BL, 1536], F32)
        taps["t_wordT"] = dout("t_wordT", [128, 4, B], F32)

    RG = [list(range(NCORES))]

    with tile.TileContext(nc) as tc:
        with (
            tc.tile_pool(name="persist", bufs=1) as pp,
            tc.tile_pool(name="pst", bufs=2, space="PSUM") as pst,
            tc.tile_pool(name="dram", bufs=1, space="DRAM") as dpool,
        ):
            # ---------- persistent smalls ----------
            ident_f = pp.tile([128, 128], F32)
            nc.sync.dma_start(ident_f[:], ident_f_d.ap())
            ident_b = pp.tile([128, 128], BF16)
            nc.sync.dma_start(ident_b[:], ident_b_d.ap())
            ones_b = pp.tile([1, 128], BF16)
            nc.sync.dma_start(ones_b[:], ones_b_d.ap())
            ones_f = pp.tile([1, 128], F32R)
            nc.sync.dma_start(ones_f[:], ones_f_d.ap())

            h0_sb = pp.tile([128, H], F32)
            nc.sync.dma_start(h0_sb[:], h0_d.ap())
            c0_sb = pp.tile([128, H], F32)
            nc.sync.dma_start(c0_sb[:], c0_d.ap())

            scores_sb = pp.tile([128, 4, 36], F32)
            e_sb = pp.tile([128, 4, 36], F32)
            rsum_sb = pp.tile([128, 4], F32)
            rinv_sb = pp.tile([128, 4], F32)
            attn_sb = pp.tile([128, 4, 512], F32)   # oss uses first 300 cols
            h_sb = pp.tile([128, H], F32)
            c_sb = pp.tile([128, H], F32)
            hT_b = pp.tile([128, 4, 128], BF16)

            # ================= attention phase =================
            with (
                tc.tile_pool(name="featsA", bufs=1) as pfA,
                tc.tile_pool(name="featsB", bufs=1) as pfB,
                tc.tile_pool(name="attnw", bufs=1) as pw_pool,
                tc.tile_pool(name="aring", bufs=3) as pr,
                tc.tile_pool(name="psu", bufs=2, space="PSUM") as psu,
            ):
                # h0^T (bf16) for Whb
                h0T_b = pw_pool.tile([128, 4, 128], BF16)
                for k in range(4):
                    pt = pst.tile([128, 128], F32, tag="tpsum")
                    nc.tensor.transpose(pt[:], h0_sb[:, k * 128:(k + 1) * 128],
                                        ident_f[:])
                    nc.scalar.copy(h0T_b[:, k, :], pt[:])

                wWw_sb = pw_pool.tile([128, 4, 512], BF16)
                nc.sync.dma_start(
                    wWw_sb[:], wWw_d.ap().rearrange("(k p) n -> p k n", p=128))

                attb_sb = []
                attw_rep = []
                for t in range(4):
                    bt = pw_pool.tile([1, 512], BF16, name=f"attb{t}")
                    nc.sync.dma_start(bt[:], attb_d.ap()[t:t + 1, :])
                    attb_sb.append(bt)
                    wrow = pw_pool.tile([1, 512], BF16, name=f"attwrow{t}")
                    nc.sync.dma_start(wrow[:], attw_d.ap()[t:t + 1, :])
                    wr = pw_pool.tile([128, 512], BF16, name=f"attwrep{t}")
                    nc.gpsimd.partition_broadcast(wr[:], wrow[:])
                    attw_rep.append(wr)

                # Whb_t = h0 @ W_w.T + b_t  (bf16)
                whb_sb = []
                for t in range(4):
                    pw = pst.tile([128, 512], F32, tag="tpsum")
                    for k in range(4):
                        nc.tensor.matmul(pw[:], h0T_b[:, k, :], wWw_sb[:, k, :],
                                         start=(k == 0), stop=False)
                    nc.tensor.matmul(pw[:], ones_b[:], attb_sb[t][:],
                                     start=False, stop=True)
                    wt = pw_pool.tile([128, 512], BF16, name=f"whb{t}")
                    nc.scalar.copy(wt[:], pw[:])
                    whb_sb.append(wt)

                wU_sb = {}
                for name, N, D, ks, _slot in ATT:
                    kn = len(ks)
                    wt = pw_pool.tile([128, kn, 512], BF16, name=f"wU{name}")
                    nc.sync.dma_start(
                        wt[:], wU_d[name].ap().rearrange("(k p) n -> p k n",
                                                         p=128))
                    wU_sb[name] = wt

                for ti, (name, N, D, ks, slot) in enumerate(ATT):
                    kn = len(ks)
                    pool_t = pfA if slot == "A" else pfB
                    feats = pool_t.tile([128, N, D], F32, tag=f"feats{slot}",
                                        name=f"feats_{name}")
                    nch = 4
                    step = _ceil(N, nch)
                    for c0i in range(0, N, step):
                        c1 = min(N, c0i + step)
                        nc.sync.dma_start(feats[:, c0i:c1, :],
                                          feats_d[name].ap()[:, c0i:c1, :])

                    for n in range(N):
                        lt = pr.tile([128, kn, 128], BF16, tag="lhsT")
                        for k in range(kn):
                            kw = ks[k]
                            ptp = pst.tile([128, 128], F32, tag="tpsum")
                            nc.tensor.transpose(
                                ptp[:kw, :],
                                feats[:, n, k * 128:k * 128 + kw],
                                ident_f[:])
                            nc.scalar.copy(lt[:kw, k, :], ptp[:kw, :])
                        pu = psu.tile([128, 512], F32, tag="upsum")
                        for k in range(kn):
                            kw = ks[k]
                            nc.tensor.matmul(pu[:], lt[:kw, k, :],
                                             wU_sb[name][:kw, k, :],
                                             start=(k == 0), stop=False)
                        nc.tensor.matmul(pu[:], ident_b[:], whb_sb[ti][:],
                                         start=False, stop=True)
                        tn = pr.tile([128, 512], BF16, tag="tanh")
                        nc.scalar.activation(tn[:], pu[:], AF.Tanh)
                        sc = pr.tile([128, 512], BF16, tag="scscratch")
                        nc.vector.scalar_tensor_tensor(
                            sc[:], tn[:], 1.0, attw_rep[ti][:],
                            op0=OP.mult, op1=OP.mult,
                            accum_out=scores_sb[:, ti, n:n + 1])

                    # softmax (no max subtraction; scores are small)
                    nc.scalar.activation(e_sb[:, ti, 0:N],
                                         scores_sb[:, ti, 0:N], AF.Exp,
                                         accum_out=rsum_sb[:, ti:ti + 1])
                    nc.vector.reciprocal(rinv_sb[:, ti:ti + 1],
                                         rsum_sb[:, ti:ti + 1])
                    acc = [pr.tile([128, D], F32, tag=f"acc{p}",
                                   name=f"acc{p}_{name}") for p in (0, 1)]
                    for n in range(N):
                        if n == 0:
                            nc.vector.tensor_scalar_mul(
                                acc[0][:], feats[:, 0, :], e_sb[:, ti, 0:1])
                        else:
                            nc.vector.scalar_tensor_tensor(
                                acc[n % 2][:], feats[:, n, :],
                                e_sb[:, ti, n:n + 1], acc[(n + 1) % 2][:],
                                op0=OP.mult, op1=OP.add)
                    nc.vector.tensor_scalar_mul(attn_sb[:, ti, 0:D],
                                                acc[(N - 1) % 2][:],
                                                rinv_sb[:, ti:ti + 1])

            if DEBUG_TAPS:
                nc.sync.dma_start(taps["t_attn"].ap(), attn_sb[:])
                nc.sync.dma_start(taps["t_scores"].ap(), scores_sb[:])

            # ================= vis/sem/x/LSTM =================
            with (
                tc.tile_pool(name="lstmw", bufs=1) as pl,
                tc.tile_pool(name="lstms", bufs=2) as ps_,
                tc.tile_pool(name="gstream", bufs=3) as pg,
                tc.tile_pool(name="psl", bufs=2, space="PSUM") as psl,
            ):
                embed_sb = pl.tile([128, E], F32)
                nc.sync.dma_start(embed_sb[:], embed_d.ap())
                asem_sb = pl.tile([128, S], F32)
                nc.sync.dma_start(asem_sb[:], asem_d.ap())
                vsem_sb = pl.tile([128, S], F32)
                nc.sync.dma_start(vsem_sb[:], vsem_d.ap())
                visb_sb = pl.tile([1, 512], F32R)
                nc.sync.dma_start(visb_sb[:], visb_d.ap())
                semb_sb = pl.tile([1, 512], F32R)
                nc.sync.dma_start(semb_sb[:], semb_d.ap())
                gbias_sb = []
                for t in range(4):
                    gb = pl.tile([1, 512], F32R, name=f"gbias{t}")
                    nc.sync.dma_start(gb[:], gbias_d.ap()[t:t + 1, :])
                    gbias_sb.append(gb)
                x_sb = pl.tile([128, 1536], F32)
                h0T_f = pl.tile([128, 4, 128], F32R)
                for k in range(4):
                    ptp = pst.tile([128, 128], F32, tag="tpsum")
                    nc.tensor.transpose(ptp[:], h0_sb[:, k * 128:(k + 1) * 128],
                                        ident_f[:])
                    nc.scalar.copy(h0T_f[:, k, :], ptp[:])

                wVis_sb = pl.tile([128, 12, 512], F32R)
                nc.sync.dma_start(wVis_sb[:],
                                  wVis_d.ap().rearrange("(k p) n -> p k n",
                                                        p=128))
                wSem_sb = pl.tile([128, 9, 512], F32R)
                nc.sync.dma_start(wSem_sb[:],
                                  wSem_d.ap().rearrange("(k p) n -> p k n",
                                                        p=128))

                # concatT_vis: [video, motion, objects] transposed (f32)
                ctv = pl.tile([128, 12, 128], F32R)
                for si, ti in enumerate((2, 1, 0)):
                    for k in range(4):
                        ptp = pst.tile([128, 128], F32, tag="tpsum")
                        nc.tensor.transpose(
                            ptp[:], attn_sb[:, ti, k * 128:(k + 1) * 128],
                            ident_f[:])
                        nc.scalar.copy(ctv[:, si * 4 + k, :], ptp[:])
                # concatT_sem: [attn_oss, asem, vsem] each seg padded to 384
                cts = pl.tile([128, 9, 128], F32R)
                sem_srcs = [attn_sb[:, 3, 0:300], asem_sb[:], vsem_sb[:]]
                sem_ks = [128, 128, 44]
                for si in range(3):
                    src = sem_srcs[si]
                    for k in range(3):
                        kw = sem_ks[k]
                        ptp = pst.tile([128, 128], F32, tag="tpsum")
                        nc.tensor.transpose(ptp[:kw, :],
                                            src[:, k * 128:k * 128 + kw],
                                            ident_f[:])
                        nc.scalar.copy(cts[:kw, si * 3 + k, :], ptp[:kw, :])

                # visual / sem  (fp32r)
                pv = psl.tile([128, 512], F32, tag="vspsum")
                for k in range(12):
                    nc.tensor.matmul(pv[:], ctv[:, k, :],
                                     wVis_sb[:, k, :],
                                     start=(k == 0), stop=False)
                nc.tensor.matmul(pv[:], ones_f[:],
                                 visb_sb[:],
                                 start=False, stop=True)
                nc.scalar.copy(x_sb[:, 0:512], pv[:])
                psm = psl.tile([128, 512], F32, tag="vspsum")
                for k in range(9):
                    kw = sem_ks[k % 3]
                    nc.tensor.matmul(psm[:], cts[:kw, k, :],
                                     wSem_sb[:kw, k, :],
                                     start=(k == 0), stop=False)
                nc.tensor.matmul(psm[:], ones_f[:],
                                 semb_sb[:],
                                 start=False, stop=True)
                nc.scalar.copy(x_sb[:, 512:1024], psm[:])
                nc.vector.tensor_copy(x_sb[:, 1024:1536], embed_sb[:])
                if DEBUG_TAPS:
                    nc.sync.dma_start(taps["t_x"].ap(), x_sb[:])

                xT = pl.tile([128, 12, 128], F32R)
                for k in range(12):
                    ptp = pst.tile([128, 128], F32, tag="tpsum")
                    nc.tensor.transpose(ptp[:], x_sb[:, k * 128:(k + 1) * 128],
                                        ident_f[:])
                    nc.scalar.copy(xT[:, k, :], ptp[:])

                # gates (fp32r, k-outer so W_ih streams)
                pgates = psl.tile([128, 4, 512], F32, tag="gatepsum", bufs=1)
                for k in range(12):
                    wk = pg.tile([128, 2048], F32R, tag="wih")
                    nc.sync.dma_start(wk[:],
                                      wIh_d.ap()[k * 128:(k + 1) * 128, :])
                    for ng in range(4):
                        nc.tensor.matmul(pgates[:, ng, :],
                                         xT[:, k, :],
                                         wk[:, ng * 512:(ng + 1) * 512]
                                         ,
                                         start=(k == 0), stop=False)
                for k in range(4):
                    wk = pg.tile([128, 2048], F32R, tag="wih")
                    nc.sync.dma_start(wk[:],
                                      wHh_d.ap()[k * 128:(k + 1) * 128, :])
                    for ng in range(4):
                        nc.tensor.matmul(pgates[:, ng, :],
                                         h0T_f[:, k, :],
                                         wk[:, ng * 512:(ng + 1) * 512]
                                         ,
                                         start=False, stop=False)
                for ng in range(4):
                    nc.tensor.matmul(pgates[:, ng, :], ones_f[:],
                                     gbias_sb[ng][:],
                                     start=False, stop=True)

                # LSTM cell elementwise (gate order i,f,g,o)
                sig_i = ps_.tile([128, 512], F32, tag="ew")
                sig_f = ps_.tile([128, 512], F32, tag="ew")
                tan_g = ps_.tile([128, 512], F32, tag="ew")
                sig_o = ps_.tile([128, 512], F32, tag="ew")
                nc.scalar.activation(sig_i[:], pgates[:, 0, :], AF.Sigmoid)
                nc.scalar.activation(sig_f[:], pgates[:, 1, :], AF.Sigmoid)
                nc.scalar.activation(tan_g[:], pgates[:, 2, :], AF.Tanh)
                nc.scalar.activation(sig_o[:], pgates[:, 3, :], AF.Sigmoid)
                t1 = ps_.tile([128, 512], F32, tag="ew2")
                t2 = ps_.tile([128, 512], F32, tag="ew2")
                nc.vector.tensor_tensor(t1[:], sig_f[:], c0_sb[:], op=OP.mult)
                nc.vector.tensor_tensor(t2[:], sig_i[:], tan_g[:], op=OP.mult)
                nc.vector.tensor_tensor(c_sb[:], t1[:], t2[:], op=OP.add)
                tc_t = ps_.tile([128, 512], F32, tag="ew2")
                nc.scalar.activation(tc_t[:], c_sb[:], AF.Tanh)
                nc.vector.tensor_tensor(h_sb[:], sig_o[:], tc_t[:], op=OP.mult)

                nc.sync.dma_start(out_h.ap(), h_sb[:])
                nc.sync.dma_start(out_c.ap(), c_sb[:])

                for k in range(4):
                    ptp = pst.tile([128, 128], F32, tag="tpsum")
                    nc.tensor.transpose(ptp[:], h_sb[:, k * 128:(k + 1) * 128],
                                        ident_f[:])
                    nc.scalar.copy(hT_b[:, k, :], ptp[:])

            # ================= AllGather h^T =================
            hT_in = dpool.tile([4, 128, 128], BF16)
            hT_all = dpool.tile([NCORES, 4, 128, 128], BF16)
            nc.sync.dma_start(hT_in[:].rearrange("k h b -> h k b"), hT_b[:])
            nc.gpsimd.collective_compute(
                "AllGather", OP.bypass, replica_groups=RG,
                ins=[hT_in.opt()], outs=[hT_all.opt()])

            # ================= phase 2 =================
            with (
                tc.tile_pool(name="ph2", bufs=1) as p2,
                tc.tile_pool(name="lgstream", bufs=2) as plg,
                tc.tile_pool(name="oring", bufs=4) as por,
                tc.tile_pool(name="ps2", bufs=2, space="PSUM") as ps2,
            ):
                hT_all_sb = p2.tile([128, NCORES, 4, 128], BF16)
                nc.sync.dma_start(
                    hT_all_sb[:], hT_all[:].rearrange("r k h b -> h r k b"))
                wTw_sb = p2.tile([128, 4, 512], BF16)
                nc.sync.dma_start(wTw_sb[:],
                                  wTw_d.ap().rearrange("(k p) n -> p k n",
                                                       p=128))
                twb_sb = p2.tile([128, 4], F32)
                nc.sync.dma_start(twb_sb[:], twb_d.ap())
                wordT = p2.tile([128, 4, B], BF16)
                for m in range(4):
                    for bg in range(2):
                        pwd = ps2.tile([128, 512], F32, tag="wpsum")
                        for k in range(4):
                            nc.tensor.matmul(
                                pwd[:],
                                wTw_sb[:, k, m * 128:(m + 1) * 128],
                                hT_all_sb[:, bg * 4:(bg + 1) * 4, k, :],
                                start=(k == 0), stop=(k == 3))
                        nc.scalar.activation(
                            wordT[:, m, bg * 512:(bg + 1) * 512], pwd[:],
                            AF.Identity, bias=twb_sb[:, m:m + 1])
                if DEBUG_TAPS:
                    nc.sync.dma_start(
                        taps["t_wordT"].ap().rearrange("p m b -> p (m b)"),
                        wordT[:].rearrange("p m b -> p (m b)"))

                lb_rep = p2.tile([128, VL], BF16)
                nc.sync.dma_start(lb_rep[:], lb_d.ap().broadcast_to((128, VL)))

                stash = [p2.tile([128, VL], BF16, name=f"stash{m}")
                         for m in range(NCORES)]
                partials = p2.tile([128, NCORES, len(N_TILES)], F32)
                sumloc = p2.tile([128, NCORES], F32)
                neg_lse = p2.tile([128, NCORES], F32)

                stats_in = [dpool.tile([128, 4], F32, name=f"stats_in{i}")
                            for i in range(2)]
                stats_out = [dpool.tile([128, 4], F32, name=f"stats_out{i}")
                             for i in range(2)]

                for half in range(2):
                    ms = range(half * 4, half * 4 + 4)
                    for (v0, vw) in N_TILES:
                        wlg = plg.tile([128, 4, 512], BF16, tag="wlg")
                        nc.sync.dma_start(
                            wlg[:, :, 0:vw],
                            wLg_d.ap()[:, v0:v0 + vw].rearrange(
                                "(k p) n -> p k n", p=128))
                        ni = v0 // 512
                        for m in ms:
                            plt = ps2.tile([128, 512], F32, tag="lpsum",
                                           bufs=4)
                            for k in range(4):
                                nc.tensor.matmul(
                                    plt[:, 0:vw],
                                    wordT[:, k, m * 128:(m + 1) * 128],
                                    wlg[:, k, 0:vw],
                                    start=(k == 0), stop=(k == 3))
                            nc.vector.scalar_tensor_tensor(
                                stash[m][:, v0:v0 + vw], plt[:, 0:vw], 1.0,
                                lb_rep[:, v0:v0 + vw],
                                op0=OP.mult, op1=OP.add)
                            esc = por.tile([128, 512], BF16, tag="esc")
                            nc.scalar.activation(
                                esc[:, 0:vw], stash[m][:, v0:v0 + vw], AF.Exp,
                                accum_out=partials[:, m, ni:ni + 1])
                    for m in ms:
                        nc.vector.tensor_reduce(
                            sumloc[:, m:m + 1], partials[:, m, :],
                            axis=mybir.AxisListType.X, op=OP.add)
                    nc.sync.dma_start(stats_in[half][:],
                                      sumloc[:, half * 4:half * 4 + 4])
                    nc.gpsimd.collective_compute(
                        "AllReduce", OP.add, replica_groups=RG,
                        ins=[stats_in[half].opt()],
                        outs=[stats_out[half].opt()])

                for half in range(2):
                    gsum = p2.tile([128, 4], F32, name=f"gsum{half}")
                    nc.sync.dma_start(gsum[:], stats_out[half][:])
                    nc.scalar.activation(neg_lse[:, half * 4:half * 4 + 4],
                                         gsum[:], AF.Ln)
                    nc.vector.tensor_scalar_mul(
                        neg_lse[:, half * 4:half * 4 + 4],
                        neg_lse[:, half * 4:half * 4 + 4], -1.0)
                    for m in range(half * 4, half * 4 + 4):
                        for ci, (o0, ow) in enumerate(OCH):
                            ob = por.tile([128, 1048], F32, tag="obuf")
                            if ci % 2 == 0:
                                nc.scalar.activation(
                                    ob[:, 0:ow], stash[m][:, o0:o0 + ow],
                                    AF.Identity, bias=neg_lse[:, m:m + 1])
                            else:
                                nc.vector.tensor_scalar_add(
                                    ob[:, 0:ow], stash[m][:, o0:o0 + ow],
                                    neg_lse[:, m:m + 1])
                            nc.sync.dma_start(
                                out_logp.ap()[m * 128:(m + 1) * 128,
                                              o0:o0 + ow],
                                ob[:, 0:ow])

    nc.compile()
    return nc


_NC_CACHE = {}


def _get_program():
    if "nc" not in _NC_CACHE:
        _NC_CACHE["nc"] = build_program()
    return _NC_CACHE["nc"]


def _prep_shared(inp):
    bf = ml_dtypes.bfloat16
    f32 = np.float32

    def T(a):
        return np.ascontiguousarray(np.asarray(a, f32).T)

    shared = {}
    shared["ident_f"] = np.eye(128, dtype=f32)
    shared["ident_b"] = np.eye(128, dtype=f32).astype(bf)
    shared["ones_f"] = np.ones((1, 128), f32)
    shared["ones_b"] = np.ones((1, 128), f32).astype(bf)
    shared["wUo"] = T(inp["Uo_w"]).astype(bf)
    shared["wUm"] = T(inp["Um_w"]).astype(bf)
    shared["wUv"] = T(inp["Uv_w"]).astype(bf)
    wuos = np.zeros((384, 512), f32)
    wuos[:300] = T(inp["Uos_w"])
    shared["wUos"] = wuos.astype(bf)
    shared["wWw"] = T(inp["W_w"]).astype(bf)
    shared["attb"] = np.stack([inp["bo"], inp["bm"], inp["bv"],
                               inp["bos"]]).astype(bf)
    shared["attw"] = np.stack([inp["wo_w"], inp["wm_w"], inp["wv_w"],
                               inp["wos_w"]]).astype(bf)
    shared["wVis"] = T(inp["vis_w"])
    shared["visb"] = np.asarray(inp["vis_b"], f32).reshape(1, 512)
    semT = T(inp["sem_w"])          # [900, 512]
    wsem = np.zeros((1152, 512), f32)
    for s in range(3):
        wsem[s * 384:s * 384 + 300] = semT[s * 300:(s + 1) * 300]
    shared["wSem"] = wsem
    shared["semb"] = np.asarray(inp["sem_b"], f32).reshape(1, 512)
    shared["wIh"] = T(inp["W_ih"])
    shared["wHh"] = T(inp["W_hh"])
    shared["gbias"] = (np.asarray(inp["b_ih"], f32)
                       + np.asarray(inp["b_hh"], f32)).reshape(4, 512)
    shared["wTw"] = T(inp["to_word_w"]).astype(bf)
    shared["twb"] = np.ascontiguousarray(
        np.asarray(inp["to_word_b"], f32).reshape(4, 128).T)
    return shared


def _prep_percore(inp, c, shared, lw_pad, lb_pad):
    bf = ml_dtypes.bfloat16
    f32 = np.float32
    b0, b1 = c * BL, (c + 1) * BL
    m = dict(shared)
    m["obj"] = np.ascontiguousarray(np.asarray(inp["objects"], f32)[b0:b1])
    m["mot"] = np.ascontiguousarray(np.asarray(inp["action"], f32)[b0:b1])
    m["vid"] = np.ascontiguousarray(np.asarray(inp["video"], f32)[b0:b1])
    m["oss"] = np.ascontiguousarray(
        np.asarray(inp["object_semantics"], f32)[b0:b1])
    m["asem"] = np.ascontiguousarray(
        np.asarray(inp["action_semantics"], f32)[b0:b1])
    m["vsem"] = np.ascontiguousarray(
        np.asarray(inp["video_semantics"], f32)[b0:b1])
    m["embed"] = np.ascontiguousarray(np.asarray(inp["embed"], f32)[b0:b1])
    m["h0"] = np.ascontiguousarray(np.asarray(inp["h0"], f32)[0, b0:b1])
    m["c0"] = np.ascontiguousarray(np.asarray(inp["c0"], f32)[0, b0:b1])

    v0, v1 = c * VL, (c + 1) * VL
    m["wLg"] = np.ascontiguousarray(lw_pad[v0:v1].T).astype(bf)
    m["lb"] = lb_pad[v0:v1].reshape(1, VL).astype(bf)
    return m


def kernel(**inputs):
    nc = _get_program()
    shared = _prep_shared(inputs)
    f32 = np.float32
    lw = np.asarray(inputs["logit_w"], f32)
    lb = np.asarray(inputs["logit_b"], f32)
    lw_pad = np.zeros((VPAD, E), f32)
    lw_pad[:V] = lw
    lb_pad = np.full((VPAD,), NEG_BIG, f32)
    lb_pad[:V] = lb
    in_maps = [_prep_percore(inputs, c, shared, lw_pad, lb_pad)
               for c in range(NCORES)]
    res = bass_utils.run_bass_kernel_spmd(nc, in_maps,
                                          core_ids=list(range(NCORES)))
    outs = res.results
    logp = np.concatenate([outs[c]["out_logp"] for c in range(NCORES)],
                          axis=1)[:, :V]
    h = np.concatenate([outs[c]["out_h"] for c in range(NCORES)], axis=0)
    c_ = np.concatenate([outs[c]["out_c"] for c in range(NCORES)], axis=0)
    if DEBUG_TAPS:
        kernel.taps = outs
    return (np.ascontiguousarray(logp), h[None], c_[None])
